# revision 14
# baseline (speedup 1.0000x reference)
"""Bass/Trainium2 kernel for batched per-expert Linear (einsum "bni,nio->bno" + bias).

Strategy:
  - Shard the n (expert) dimension across the 8 NeuronCores: 8 experts/core.
  - Host-side layout choice: pre-transpose x to (n, d_in, batch) and cast
    x/weight to bf16 (PSUM accumulation stays fp32; measured output error
    ~2.4e-3 relative).  This puts the contraction dim (d_in) on SBUF
    partitions for both matmul operands with fully-contiguous DMAs, and
    halves input HBM traffic.
  - Per core: out[b, n, o] = sum_k xT[n, k, b] * w[n, k, o] + bias[n, o]
    as 256 PE matmuls (lhsT = xT chunk [128k, 128b] stationary,
    rhs = w tile [128k, 512o] moving, accumulate 4 k-tiles in PSUM),
    bias added on the PSUM->SBUF copy (DVE), natural-layout output DMA.
"""

import numpy as np
import ml_dtypes

import concourse.bass as bass
import concourse.bacc as bacc
import concourse.mybir as mybir
from concourse import tile
from concourse.bass_utils import run_bass_kernel_spmd

BF16 = ml_dtypes.bfloat16
N_CORES = 8
N, D_IN, D_OUT, BATCH = 64, 512, 512, 1024
NPC = N // N_CORES  # experts per core
P = 128
KT = D_IN // P  # contraction tiles
BT = BATCH // P  # batch tiles


def _build():
    nc = bacc.Bacc(None, target_bir_lowering=False)
    # SBUF-native DRAM layouts (partition dim second): every DMA moves
    # 4-8KB contiguous per partition -> minimal descriptors, max HBM eff.
    xt = nc.dram_tensor("xt", [NPC, P, KT, BATCH], mybir.dt.bfloat16, kind="ExternalInput")
    w = nc.dram_tensor("w", [NPC, P, KT, D_OUT], mybir.dt.bfloat16, kind="ExternalInput")
    bias = nc.dram_tensor("bias", [NPC, D_OUT], mybir.dt.float32, kind="ExternalInput")
    out = nc.dram_tensor("out", [NPC, P, BT, D_OUT], mybir.dt.bfloat16, kind="ExternalOutput")

    with tile.TileContext(nc) as tc:
        with (
            tc.tile_pool(name="resident", bufs=1) as resp,
            tc.tile_pool(name="outp", bufs=6) as outp,
            tc.tile_pool(name="psum", bufs=6, space="PSUM") as psump,
        ):
            # Two HWDGE issue queues (SP + ACT) share the 16 SDMA engines.
            # Keep each expert's x+w adjacent on ONE queue (alternating per
            # expert) so an expert's weights are never starved behind the
            # other experts' activations on a different queue.
            qs = [nc.sync, nc.scalar]

            xs, ws = [], []
            for n in range(NPC):
                xtt = resp.tile([P, KT, BATCH], mybir.dt.bfloat16, name=f"x{n}", tag=f"x{n}")
                wt = resp.tile([P, KT, D_OUT], mybir.dt.bfloat16, name=f"w{n}", tag=f"w{n}")
                q = qs[n % 2]
                # weights first (smaller; first matmul needs w AND x);
                # split x0 per k-tile so the very first matmul only waits
                # for w0 + a quarter of x0
                q.dma_start(wt[:], w[n])
                if n == 0:
                    for kt in range(KT):
                        q.dma_start(xtt[:, kt, :], xt[n, :, kt, :])
                else:
                    q.dma_start(xtt[:], xt[n])
                xs.append(xtt)
                ws.append(wt)

            bias_sb = resp.tile([P, NPC, D_OUT], mybir.dt.float32, name="bias_sb", tag="bias_sb")
            bias_ap = bias[:]
            bias_bcast = bass.AP(
                tensor=bias_ap.tensor,
                offset=bias_ap.offset,
                ap=[[0, P], bias_ap.ap[0], bias_ap.ap[1]],
            )
            nc.gpsimd.dma_start(out=bias_sb[:], in_=bias_bcast)

            HB = BT // 2  # half-batch store granularity
            st_i = 0
            for n in range(NPC):
                for half in range(2):
                    ob = outp.tile([P, HB, D_OUT], mybir.dt.bfloat16, name="ob")
                    for hb in range(HB):
                        bt = half * HB + hb
                        ps = psump.tile([P, D_OUT], mybir.dt.float32, name="ps")
                        for kt in range(KT):
                            nc.tensor.matmul(
                                ps[:],
                                xs[n][:, kt, bass.ts(bt, P)],
                                ws[n][:, kt, :],
                                start=(kt == 0),
                                stop=(kt == KT - 1),
                            )
                        nc.vector.tensor_add(ob[:, hb, :], ps[:], bias_sb[:, n, :])
                    dst = out[n, :, bass.ts(half, HB), :]
                    qs[st_i % 2].dma_start(dst, ob[:])
                    st_i += 1
    nc.compile()
    return nc


_NC = None


def _get_nc():
    global _NC
    if _NC is None:
        _NC = _build()
    return _NC


def _run(x, weight, bias, **run_kwargs):
    # xt[n, p, kt, b] = x[b, n, kt*128+p]
    xt_full = (
        x.transpose(1, 2, 0).reshape(N, KT, P, BATCH).transpose(0, 2, 1, 3).astype(BF16)
    )
    # w_dev[n, p, kt, o] = weight[n, kt*128+p, o]
    w_dev = weight.reshape(N, KT, P, D_OUT).transpose(0, 2, 1, 3).astype(BF16)
    bias = np.ascontiguousarray(bias, dtype=np.float32)
    in_maps = []
    for c in range(N_CORES):
        sl = slice(c * NPC, (c + 1) * NPC)
        in_maps.append(
            {
                "xt": np.ascontiguousarray(xt_full[sl]),
                "w": np.ascontiguousarray(w_dev[sl]),
                "bias": np.ascontiguousarray(bias[sl]),
            }
        )
    res = run_bass_kernel_spmd(_get_nc(), in_maps, core_ids=list(range(N_CORES)), **run_kwargs)
    out = np.empty((BATCH, N, D_OUT), dtype=np.float32)
    for c in range(N_CORES):
        # device out[n, p, bt, o] -> out[bt*128+p, n_global, o]
        r = res.results[c]["out"].transpose(2, 1, 0, 3).reshape(BATCH, NPC, D_OUT)
        out[:, c * NPC : (c + 1) * NPC, :] = r.astype(np.float32)
    return out, res


def kernel(x, weight, bias):
    out, _ = _run(x, weight, bias)
    return out


# revision 17
# speedup vs baseline: 1.0873x; 1.0873x over previous
"""Bass/Trainium2 kernel for batched per-expert Linear (einsum "bni,nio->bno" + bias).

Strategy:
  - Shard the n (expert) dimension across the 8 NeuronCores: 8 experts/core.
  - Host-side layout choice: pre-transpose x to (n, d_in, batch) and cast
    x/weight to bf16 (PSUM accumulation stays fp32; measured output error
    ~2.4e-3 relative).  This puts the contraction dim (d_in) on SBUF
    partitions for both matmul operands with fully-contiguous DMAs, and
    halves input HBM traffic.
  - Per core: out[b, n, o] = sum_k xT[n, k, b] * w[n, k, o] + bias[n, o]
    as 256 PE matmuls (lhsT = xT chunk [128k, 128b] stationary,
    rhs = w tile [128k, 512o] moving, accumulate 4 k-tiles in PSUM),
    bias added on the PSUM->SBUF copy (DVE), natural-layout output DMA.
"""

import numpy as np
import ml_dtypes

import concourse.bass as bass
import concourse.bacc as bacc
import concourse.mybir as mybir
from concourse import tile
from concourse.bass_utils import run_bass_kernel_spmd

BF16 = ml_dtypes.bfloat16
N_CORES = 8
N, D_IN, D_OUT, BATCH = 64, 512, 512, 1024
NPC = N // N_CORES  # experts per core
P = 128
KT = D_IN // P  # contraction tiles
BT = BATCH // P  # batch tiles


def _build():
    nc = bacc.Bacc(None, target_bir_lowering=False)
    # SBUF-native DRAM layouts (partition dim second): every DMA moves
    # 4-8KB contiguous per partition -> minimal descriptors, max HBM eff.
    xt = nc.dram_tensor("xt", [NPC, P, KT, BATCH], mybir.dt.bfloat16, kind="ExternalInput")
    w = nc.dram_tensor("w", [NPC, P, KT, D_OUT], mybir.dt.bfloat16, kind="ExternalInput")
    # bias host-replicated across partitions: clean HWDGE load instead of a
    # SWDGE broadcast that hogs the DMA engines during the critical ramp
    bias = nc.dram_tensor("bias", [P, NPC, D_OUT], mybir.dt.float32, kind="ExternalInput")
    out = nc.dram_tensor("out", [NPC, P, BT, D_OUT], mybir.dt.bfloat16, kind="ExternalOutput")

    with tile.TileContext(nc) as tc:
        with (
            tc.tile_pool(name="resident", bufs=1) as resp,
            tc.tile_pool(name="outp", bufs=6) as outp,
            tc.tile_pool(name="psum", bufs=6, space="PSUM") as psump,
        ):
            # Two HWDGE issue queues (SP + ACT) share the 16 SDMA engines.
            # Keep each expert's x+w adjacent on ONE queue (alternating per
            # expert) so an expert's weights are never starved behind the
            # other experts' activations on a different queue.
            qs = [nc.sync, nc.scalar]

            bias_sb = resp.tile([P, NPC, D_OUT], mybir.dt.float32, name="bias_sb", tag="bias_sb")
            xs, ws = [], []
            for n in range(NPC):
                xtt = resp.tile([P, KT, BATCH], mybir.dt.bfloat16, name=f"x{n}", tag=f"x{n}")
                wt = resp.tile([P, KT, D_OUT], mybir.dt.bfloat16, name=f"w{n}", tag=f"w{n}")
                q = qs[n % 2]
                # weights first: the first matmul of expert n needs w fully
                # and only the first slice of x
                q.dma_start(wt[:], w[n])
                q.dma_start(xtt[:], xt[n])
                xs.append(xtt)
                ws.append(wt)
                if n == 0:
                    # after each queue's first expert: half the bias each
                    nc.sync.dma_start(bias_sb[:, : NPC // 2, :], bias[:, : NPC // 2, :])
                    nc.scalar.dma_start(bias_sb[:, NPC // 2 :, :], bias[:, NPC // 2 :, :])

            HB = BT // 2  # half-batch store granularity
            st_i = 0
            for n in range(NPC):
                for half in range(2):
                    ob = outp.tile([P, HB, D_OUT], mybir.dt.bfloat16, name="ob")
                    for hb in range(HB):
                        bt = half * HB + hb
                        ps = psump.tile([P, D_OUT], mybir.dt.float32, name="ps")
                        for kt in range(KT):
                            nc.tensor.matmul(
                                ps[:],
                                xs[n][:, kt, bass.ts(bt, P)],
                                ws[n][:, kt, :],
                                start=(kt == 0),
                                stop=(kt == KT - 1),
                            )
                        nc.vector.tensor_add(ob[:, hb, :], ps[:], bias_sb[:, n, :])
                    dst = out[n, :, bass.ts(half, HB), :]
                    qs[st_i % 2].dma_start(dst, ob[:])
                    st_i += 1
    nc.compile()
    return nc


_NC = None


def _get_nc():
    global _NC
    if _NC is None:
        _NC = _build()
    return _NC


def _run(x, weight, bias, **run_kwargs):
    # xt[n, p, kt, b] = x[b, n, kt*128+p]
    xt_full = (
        x.transpose(1, 2, 0).reshape(N, KT, P, BATCH).transpose(0, 2, 1, 3).astype(BF16)
    )
    # w_dev[n, p, kt, o] = weight[n, kt*128+p, o]
    w_dev = weight.reshape(N, KT, P, D_OUT).transpose(0, 2, 1, 3).astype(BF16)
    bias_rep = np.ascontiguousarray(
        np.broadcast_to(bias.astype(np.float32)[None], (P, N, D_OUT)).transpose(1, 0, 2)
    )  # (N, P, D_OUT) -> sliced per core then fed as [P, NPC, D_OUT]
    in_maps = []
    for c in range(N_CORES):
        sl = slice(c * NPC, (c + 1) * NPC)
        in_maps.append(
            {
                "xt": np.ascontiguousarray(xt_full[sl]),
                "w": np.ascontiguousarray(w_dev[sl]),
                "bias": np.ascontiguousarray(bias_rep[sl].transpose(1, 0, 2)),
            }
        )
    res = run_bass_kernel_spmd(_get_nc(), in_maps, core_ids=list(range(N_CORES)), **run_kwargs)
    out = np.empty((BATCH, N, D_OUT), dtype=np.float32)
    for c in range(N_CORES):
        # device out[n, p, bt, o] -> out[bt*128+p, n_global, o]
        r = res.results[c]["out"].transpose(2, 1, 0, 3).reshape(BATCH, NPC, D_OUT)
        out[:, c * NPC : (c + 1) * NPC, :] = r.astype(np.float32)
    return out, res


def kernel(x, weight, bias):
    out, _ = _run(x, weight, bias)
    return out


# revision 18
# speedup vs baseline: 1.1163x; 1.0266x over previous
"""Bass/Trainium2 kernel for batched per-expert Linear (einsum "bni,nio->bno" + bias).

Strategy:
  - Shard the n (expert) dimension across the 8 NeuronCores: 8 experts/core.
  - Host-side layout choice: pre-transpose x to (n, d_in, batch) and cast
    x/weight to bf16 (PSUM accumulation stays fp32; measured output error
    ~2.4e-3 relative).  This puts the contraction dim (d_in) on SBUF
    partitions for both matmul operands with fully-contiguous DMAs, and
    halves input HBM traffic.
  - Per core: out[b, n, o] = sum_k xT[n, k, b] * w[n, k, o] + bias[n, o]
    as 256 PE matmuls (lhsT = xT chunk [128k, 128b] stationary,
    rhs = w tile [128k, 512o] moving, accumulate 4 k-tiles in PSUM),
    bias added on the PSUM->SBUF copy (DVE), natural-layout output DMA.
"""

import numpy as np
import ml_dtypes

import concourse.bass as bass
import concourse.bacc as bacc
import concourse.mybir as mybir
from concourse import tile
from concourse.bass_utils import run_bass_kernel_spmd

BF16 = ml_dtypes.bfloat16
N_CORES = 8
N, D_IN, D_OUT, BATCH = 64, 512, 512, 1024
NPC = N // N_CORES  # experts per core
P = 128
KT = D_IN // P  # contraction tiles
BT = BATCH // P  # batch tiles


def _build():
    nc = bacc.Bacc(None, target_bir_lowering=False)
    # SBUF-native DRAM layouts (partition dim second): every DMA moves
    # 4-8KB contiguous per partition -> minimal descriptors, max HBM eff.
    xt = nc.dram_tensor("xt", [NPC, P, KT, BATCH], mybir.dt.bfloat16, kind="ExternalInput")
    w = nc.dram_tensor("w", [NPC, P, KT, D_OUT], mybir.dt.bfloat16, kind="ExternalInput")
    # bias host-replicated across partitions: clean HWDGE load instead of a
    # SWDGE broadcast that hogs the DMA engines during the critical ramp
    bias = nc.dram_tensor("bias", [P, NPC, D_OUT], mybir.dt.bfloat16, kind="ExternalInput")
    out = nc.dram_tensor("out", [NPC, P, BT, D_OUT], mybir.dt.bfloat16, kind="ExternalOutput")

    with tile.TileContext(nc) as tc:
        with (
            tc.tile_pool(name="resident", bufs=1) as resp,
            tc.tile_pool(name="outp", bufs=6) as outp,
            tc.tile_pool(name="psum", bufs=8, space="PSUM") as psump,
        ):
            # Two HWDGE issue queues (SP + ACT) share the 16 SDMA engines.
            # Keep each expert's x+w adjacent on ONE queue (alternating per
            # expert) so an expert's weights are never starved behind the
            # other experts' activations on a different queue.
            qs = [nc.sync, nc.scalar]

            bias_sb = resp.tile([P, NPC, D_OUT], mybir.dt.bfloat16, name="bias_sb", tag="bias_sb")
            xs, ws = [], []
            for n in range(NPC):
                xtt = resp.tile([P, KT, BATCH], mybir.dt.bfloat16, name=f"x{n}", tag=f"x{n}")
                wt = resp.tile([P, KT, D_OUT], mybir.dt.bfloat16, name=f"w{n}", tag=f"w{n}")
                q = qs[n % 2]
                # weights first: the first matmul of expert n needs w fully
                # and only the first slice of x
                if n == 0:
                    # split expert 0 across BOTH queues for the fastest start
                    nc.sync.dma_start(wt[:], w[n])
                    nc.sync.dma_start(xtt[:, : KT // 2, :], xt[n, :, : KT // 2, :])
                    nc.scalar.dma_start(xtt[:, KT // 2 :, :], xt[n, :, KT // 2 :, :])
                else:
                    q.dma_start(wt[:], w[n])
                    q.dma_start(xtt[:], xt[n])
                xs.append(xtt)
                ws.append(wt)
                if n == 0:
                    nc.sync.dma_start(bias_sb[:, : NPC // 2, :], bias[:, : NPC // 2, :])
                elif n == 1:
                    nc.scalar.dma_start(bias_sb[:, NPC // 2 :, :], bias[:, NPC // 2 :, :])

            HB = BT // 2  # half-batch store granularity
            st_i = 0
            for n in range(NPC):
                for half in range(2):
                    ob = outp.tile([P, HB, D_OUT], mybir.dt.bfloat16, name="ob")
                    for hb in range(HB):
                        bt = half * HB + hb
                        ps = psump.tile([P, D_OUT], mybir.dt.float32, name="ps")
                        for kt in range(KT):
                            nc.tensor.matmul(
                                ps[:],
                                xs[n][:, kt, bass.ts(bt, P)],
                                ws[n][:, kt, :],
                                start=(kt == 0),
                                stop=(kt == KT - 1),
                            )
                        nc.vector.tensor_add(ob[:, hb, :], ps[:], bias_sb[:, n, :])
                    dst = out[n, :, bass.ts(half, HB), :]
                    qs[st_i % 2].dma_start(dst, ob[:])
                    st_i += 1
    nc.compile()
    return nc


_NC = None


def _get_nc():
    global _NC
    if _NC is None:
        _NC = _build()
    return _NC


def _run(x, weight, bias, **run_kwargs):
    # xt[n, p, kt, b] = x[b, n, kt*128+p]
    xt_full = (
        x.transpose(1, 2, 0).reshape(N, KT, P, BATCH).transpose(0, 2, 1, 3).astype(BF16)
    )
    # w_dev[n, p, kt, o] = weight[n, kt*128+p, o]
    w_dev = weight.reshape(N, KT, P, D_OUT).transpose(0, 2, 1, 3).astype(BF16)
    bias_rep = np.ascontiguousarray(
        np.broadcast_to(bias.astype(BF16)[None], (P, N, D_OUT)).transpose(1, 0, 2)
    )  # (N, P, D_OUT) -> sliced per core then fed as [P, NPC, D_OUT]
    in_maps = []
    for c in range(N_CORES):
        sl = slice(c * NPC, (c + 1) * NPC)
        in_maps.append(
            {
                "xt": np.ascontiguousarray(xt_full[sl]),
                "w": np.ascontiguousarray(w_dev[sl]),
                "bias": np.ascontiguousarray(bias_rep[sl].transpose(1, 0, 2)),
            }
        )
    res = run_bass_kernel_spmd(_get_nc(), in_maps, core_ids=list(range(N_CORES)), **run_kwargs)
    out = np.empty((BATCH, N, D_OUT), dtype=np.float32)
    for c in range(N_CORES):
        # device out[n, p, bt, o] -> out[bt*128+p, n_global, o]
        r = res.results[c]["out"].transpose(2, 1, 0, 3).reshape(BATCH, NPC, D_OUT)
        out[:, c * NPC : (c + 1) * NPC, :] = r.astype(np.float32)
    return out, res


def kernel(x, weight, bias):
    out, _ = _run(x, weight, bias)
    return out


# revision 19
# speedup vs baseline: 1.2133x; 1.0869x over previous
"""Bass/Trainium2 kernel for batched per-expert Linear (einsum "bni,nio->bno" + bias).

Strategy:
  - Shard the n (expert) dimension across the 8 NeuronCores: 8 experts/core.
  - Host-side layout choice: pre-transpose x to (n, d_in, batch) and cast
    x/weight to bf16 (PSUM accumulation stays fp32; measured output error
    ~2.4e-3 relative).  This puts the contraction dim (d_in) on SBUF
    partitions for both matmul operands with fully-contiguous DMAs, and
    halves input HBM traffic.
  - Per core: out[b, n, o] = sum_k xT[n, k, b] * w[n, k, o] + bias[n, o]
    as 256 PE matmuls (lhsT = xT chunk [128k, 128b] stationary,
    rhs = w tile [128k, 512o] moving, accumulate 4 k-tiles in PSUM),
    bias added on the PSUM->SBUF copy (DVE), natural-layout output DMA.
"""

import numpy as np
import ml_dtypes

import concourse.bass as bass
import concourse.bacc as bacc
import concourse.mybir as mybir
from concourse import tile
from concourse.bass_utils import run_bass_kernel_spmd

BF16 = ml_dtypes.bfloat16
N_CORES = 8
N, D_IN, D_OUT, BATCH = 64, 512, 512, 1024
NPC = N // N_CORES  # experts per core
P = 128
KT = D_IN // P  # contraction tiles
BT = BATCH // P  # batch tiles


def _build():
    nc = bacc.Bacc(None, target_bir_lowering=False)
    # SBUF-native DRAM layouts (partition dim second): every DMA moves
    # 4-8KB contiguous per partition -> minimal descriptors, max HBM eff.
    xt = nc.dram_tensor("xt", [NPC, P, KT, BATCH], mybir.dt.bfloat16, kind="ExternalInput")
    w = nc.dram_tensor("w", [NPC, P, KT, D_OUT], mybir.dt.bfloat16, kind="ExternalInput")
    # bias host-replicated across partitions: clean HWDGE load instead of a
    # SWDGE broadcast that hogs the DMA engines during the critical ramp
    bias = nc.dram_tensor("bias", [P, NPC, D_OUT], mybir.dt.bfloat16, kind="ExternalInput")
    out = nc.dram_tensor("out", [NPC, P, BT, D_OUT], mybir.dt.bfloat16, kind="ExternalOutput")

    with tile.TileContext(nc) as tc:
        with (
            tc.tile_pool(name="resident", bufs=1) as resp,
            tc.tile_pool(name="outp", bufs=16) as outp,
            tc.tile_pool(name="psum", bufs=8, space="PSUM") as psump,
        ):
            # Two HWDGE issue queues (SP + ACT) share the 16 SDMA engines.
            # Keep each expert's x+w adjacent on ONE queue (alternating per
            # expert) so an expert's weights are never starved behind the
            # other experts' activations on a different queue.
            qs = [nc.sync, nc.scalar]

            bias_sb = resp.tile([P, NPC, D_OUT], mybir.dt.bfloat16, name="bias_sb", tag="bias_sb")
            xs, ws = [], []
            for n in range(NPC):
                xtt = resp.tile([P, KT, BATCH], mybir.dt.bfloat16, name=f"x{n}", tag=f"x{n}")
                wt = resp.tile([P, KT, D_OUT], mybir.dt.bfloat16, name=f"w{n}", tag=f"w{n}")
                q = qs[n % 2]
                # weights first: the first matmul of expert n needs w fully
                # and only the first slice of x
                if n == 0:
                    # split expert 0 across BOTH queues for the fastest start
                    nc.sync.dma_start(wt[:, : KT // 2, :], w[n, :, : KT // 2, :])
                    nc.scalar.dma_start(wt[:, KT // 2 :, :], w[n, :, KT // 2 :, :])
                    nc.sync.dma_start(xtt[:, : KT // 2, :], xt[n, :, : KT // 2, :])
                    nc.scalar.dma_start(xtt[:, KT // 2 :, :], xt[n, :, KT // 2 :, :])
                else:
                    q.dma_start(wt[:], w[n])
                    q.dma_start(xtt[:], xt[n])
                xs.append(xtt)
                ws.append(wt)
                if n == 0:
                    nc.sync.dma_start(bias_sb[:, : NPC // 2, :], bias[:, : NPC // 2, :])
                elif n == 1:
                    nc.scalar.dma_start(bias_sb[:, NPC // 2 :, :], bias[:, NPC // 2 :, :])

            HB = BT // 2  # half-batch store granularity
            st_i = 0
            for n in range(NPC):
                for half in range(2):
                    ob = outp.tile([P, HB, D_OUT], mybir.dt.bfloat16, name="ob")
                    for hb in range(HB):
                        bt = half * HB + hb
                        ps = psump.tile([P, D_OUT], mybir.dt.float32, name="ps")
                        for kt in range(KT):
                            nc.tensor.matmul(
                                ps[:],
                                xs[n][:, kt, bass.ts(bt, P)],
                                ws[n][:, kt, :],
                                start=(kt == 0),
                                stop=(kt == KT - 1),
                            )
                        nc.vector.tensor_add(ob[:, hb, :], ps[:], bias_sb[:, n, :])
                    dst = out[n, :, bass.ts(half, HB), :]
                    qs[st_i % 2].dma_start(dst, ob[:])
                    st_i += 1
    nc.compile()
    return nc


_NC = None


def _get_nc():
    global _NC
    if _NC is None:
        _NC = _build()
    return _NC


def _run(x, weight, bias, **run_kwargs):
    # xt[n, p, kt, b] = x[b, n, kt*128+p]
    xt_full = (
        x.transpose(1, 2, 0).reshape(N, KT, P, BATCH).transpose(0, 2, 1, 3).astype(BF16)
    )
    # w_dev[n, p, kt, o] = weight[n, kt*128+p, o]
    w_dev = weight.reshape(N, KT, P, D_OUT).transpose(0, 2, 1, 3).astype(BF16)
    bias_rep = np.ascontiguousarray(
        np.broadcast_to(bias.astype(BF16)[None], (P, N, D_OUT)).transpose(1, 0, 2)
    )  # (N, P, D_OUT) -> sliced per core then fed as [P, NPC, D_OUT]
    in_maps = []
    for c in range(N_CORES):
        sl = slice(c * NPC, (c + 1) * NPC)
        in_maps.append(
            {
                "xt": np.ascontiguousarray(xt_full[sl]),
                "w": np.ascontiguousarray(w_dev[sl]),
                "bias": np.ascontiguousarray(bias_rep[sl].transpose(1, 0, 2)),
            }
        )
    res = run_bass_kernel_spmd(_get_nc(), in_maps, core_ids=list(range(N_CORES)), **run_kwargs)
    out = np.empty((BATCH, N, D_OUT), dtype=np.float32)
    for c in range(N_CORES):
        # device out[n, p, bt, o] -> out[bt*128+p, n_global, o]
        r = res.results[c]["out"].transpose(2, 1, 0, 3).reshape(BATCH, NPC, D_OUT)
        out[:, c * NPC : (c + 1) * NPC, :] = r.astype(np.float32)
    return out, res


def kernel(x, weight, bias):
    out, _ = _run(x, weight, bias)
    return out
